# revision 6
# baseline (speedup 1.0000x reference)
"""Trainium2 Bass kernel for nn_Attention (B=4, N=2048, C=768, H=12).

Sharding: 8 cores = 4 batches x 2 head-groups (6 heads each).
Each core computes, for its (batch b, head-group g):
    qT/kT = (W{q,k}_g @ x_b^T)          [384, 2048]  (scale folded into Wq)
    v     = x_b @ Wv_g^T                [2048, 384]  (v_bias folded into proj bias
                                                      since softmax rows sum to 1)
    per head h, q-block: s^T = k_h^T^T.. scores computed transposed
        p = exp(s^T)  (no max-subtraction: scores ~ N(0,1))
        out^T = [v_h | 1]^T @ p   -> row 64 = softmax sums
        out_n^T = out^T[0:64] / sums
    y_partial = out_n @ Wp_g^T + pb_eff     [2048, 768]
Host sums the two partials per batch (tensor-parallel unshard).
"""

import numpy as np
import ml_dtypes

import concourse.bass as bass
import concourse.tile as tile
from concourse import bacc, mybir
from concourse.bass import ds, ts
from concourse.bass_utils import run_bass_kernel_spmd

N_CORES = 8
B, N, C = 4, 2048, 768
H, HD = 12, 64
HPC, GF = 6, 384          # heads per core, features per group
SCALE = HD ** -0.5        # 1/8, exact power of two
BF16, F32 = mybir.dt.bfloat16, mybir.dt.float32
CP = C // 128             # 6 contraction partition-tiles
FP = GF // 128            # 3 feature partition-tiles per group
QB = 512                  # q block
NB = N // QB              # 4
NT = N // 128             # 16 token tiles
KT = N // 128             # 16 k tiles
PAIR_LAG = 2
EXPF = mybir.ActivationFunctionType.Exp


def _body(nc, tc, pools, aps):
    const, qkvp, pp, normp, yp, psA, psS, psO = pools
    xT, wqT, wkT, wvT, wpT, qb, pb, out = aps

    # ---- load inputs to SBUF
    xT_sb = const.tile([128, CP, N], BF16, tag="xT")
    nc.sync.dma_start(out=xT_sb[:], in_=xT.ap().rearrange("(t p) n -> p t n", p=128))
    wq_sb = const.tile([128, CP, GF], BF16, tag="wq")
    nc.sync.dma_start(out=wq_sb[:], in_=wqT.ap().rearrange("(t p) n -> p t n", p=128))
    wk_sb = const.tile([128, CP, GF], BF16, tag="wk")
    nc.sync.dma_start(out=wk_sb[:], in_=wkT.ap().rearrange("(t p) n -> p t n", p=128))
    wv_sb = const.tile([128, CP, GF], BF16, tag="wv")
    nc.sync.dma_start(out=wv_sb[:], in_=wvT.ap().rearrange("(t p) n -> p t n", p=128))
    wp_sb = const.tile([128, FP, C], BF16, tag="wp")
    nc.sync.dma_start(out=wp_sb[:], in_=wpT.ap().rearrange("(t p) n -> p t n", p=128))
    qb_sb = const.tile([128, FP], F32, tag="qb")
    nc.sync.dma_start(out=qb_sb[:], in_=qb.ap().rearrange("(t p) -> p t", p=128))
    pb_sb = const.tile([128, C], F32, tag="pb")
    pb_ap = pb.ap()
    pb_bcast = bass.AP(tensor=pb_ap.tensor, offset=pb_ap.offset, ap=[[0, 128]] + list(pb_ap.ap))
    nc.sync.dma_start(out=pb_sb[:], in_=pb_bcast)

    qT_sb = qkvp.tile([128, FP, N], BF16, tag="qT")
    kT_sb = qkvp.tile([128, FP, N], BF16, tag="kT")
    v_sb = qkvp.tile([128, NT, HPC, HD + 1], BF16, tag="v")
    outT_sb = qkvp.tile([128, FP, N], BF16, tag="outT")

    # ones column for softmax sums
    nc.vector.memset(v_sb[:, :, :, HD], 1.0)
    ones64 = const.tile([1, 64], F32, tag="ones64")
    nc.vector.memset(ones64[:], 1.0)

    # ---- qkv projections
    def qk_block(w_sb, f, dst_sb, is_q):
        for nb in range(NB):
            ps = psA.tile([128, QB], F32, tag="mm")
            for cp in range(CP):
                nc.tensor.matmul(
                    ps[:],
                    lhsT=w_sb[:, cp, ts(f, 128)],
                    rhs=xT_sb[:, cp, ds(nb * QB, QB)],
                    start=(cp == 0),
                    stop=(cp == CP - 1),
                )
            if is_q:
                nc.vector.tensor_scalar_add(
                    dst_sb[:, f, ds(nb * QB, QB)], ps[:], qb_sb[:, f : f + 1]
                )
            else:
                nc.vector.tensor_copy(dst_sb[:, f, ds(nb * QB, QB)], ps[:])

    qk_block(wq_sb, 0, qT_sb, True)
    qk_block(wk_sb, 0, kT_sb, False)

    # v (natural layout), interleaved strided into v_sb
    for nt in range(NT):
        ps = psA.tile([128, GF], F32, tag="mm")
        for cp in range(CP):
            nc.tensor.matmul(
                ps[:],
                lhsT=xT_sb[:, cp, ts(nt, 128)],
                rhs=wv_sb[:, cp, :],
                start=(cp == 0),
                stop=(cp == CP - 1),
            )
        nc.vector.tensor_copy(
            v_sb[:, nt, :, 0:HD], ps[:].rearrange("p (h d) -> p h d", h=HPC)
        )

    for f in range(1, FP):
        qk_block(wq_sb, f, qT_sb, True)
        qk_block(wk_sb, f, kT_sb, False)

    # ---- attention, head pairs x q blocks
    for hp in range(FP):
        for nb in range(NB):
            p_t = pp.tile([128, KT, 2, QB], BF16, tag="p")
            pso = [psO.tile([128, QB], F32, tag="acc", name=f"pso{hp}_{nb}_{i}") for i in range(2)]

            def av(kt):
                for hi in range(2):
                    h = 2 * hp + hi
                    nc.tensor.matmul(
                        pso[hi][0 : HD + 1, :],
                        lhsT=v_sb[:, kt, h, :],
                        rhs=p_t[:, kt, hi, :],
                        start=(kt == 0),
                        stop=(kt == KT - 1),
                        skip_group_check=True,
                    )

            for kt in range(KT):
                ps_s = psS.tile([128, 2, QB], F32, tag="s")
                for hi in range(2):
                    po = hi * 64
                    nc.tensor.matmul(
                        ps_s[:, hi, :],
                        lhsT=kT_sb[po : po + 64, hp, ts(kt, 128)],
                        rhs=qT_sb[po : po + 64, hp, ds(nb * QB, QB)],
                        start=True,
                        stop=True,
                        skip_group_check=True,
                    )
                nc.scalar.activation(p_t[:, kt, :, :], ps_s[:], EXPF)
                if kt >= PAIR_LAG:
                    av(kt - PAIR_LAG)
            for kt in range(KT - PAIR_LAG, KT):
                av(kt)

            for hi in range(2):
                sums_row = normp.tile([1, QB], F32, tag="sumsrow")
                nc.vector.tensor_copy(sums_row[:], pso[hi][HD : HD + 1, :])
                sums_b = psS.tile([64, QB], F32, tag="s", name=f"sb{hp}_{nb}_{hi}")
                nc.tensor.matmul(
                    sums_b[:], lhsT=ones64[:], rhs=sums_row[:],
                    start=True, stop=True, skip_group_check=True,
                )
                rec_b = normp.tile([64, QB], F32, tag="rec")
                nc.vector.reciprocal_approx_fast(rec_b[:], sums_b[:])
                if hi == 0:
                    nc.vector.tensor_mul(
                        outT_sb[0:64, hp, ds(nb * QB, QB)], pso[hi][0:HD, :], rec_b[:]
                    )
                else:
                    tmp = normp.tile([64, QB], BF16, tag="tmp")
                    nc.vector.tensor_mul(tmp[:], pso[hi][0:HD, :], rec_b[:])
                    nc.gpsimd.dma_start(
                        out=outT_sb[64:128, hp, ds(nb * QB, QB)], in_=tmp[:]
                    )

    # ---- output projection + bias
    for qt in range(NT):
        psy = [psA.tile([128, GF], F32, tag="mm", name=f"psy{qt}_{i}") for i in range(2)]
        for f in range(FP):
            for oc in range(2):
                nc.tensor.matmul(
                    psy[oc][:],
                    lhsT=outT_sb[:, f, ts(qt, 128)],
                    rhs=wp_sb[:, f, ds(oc * GF, GF)],
                    start=(f == 0),
                    stop=(f == FP - 1),
                    skip_group_check=True,
                )
        y_sb = yp.tile([128, C], F32, tag="y")
        for oc in range(2):
            nc.vector.tensor_add(
                y_sb[:, ds(oc * GF, GF)], psy[oc][:], pb_sb[:, ds(oc * GF, GF)]
            )
        nc.sync.dma_start(out=out.ap()[ts(qt, 128), :], in_=y_sb[:])


def build(krep=1):
    nc = bacc.Bacc("TRN2", target_bir_lowering=False, debug=False, num_devices=N_CORES)
    xT = nc.dram_tensor("xT", [C, N], BF16, kind="ExternalInput")
    wqT = nc.dram_tensor("wqT", [C, GF], BF16, kind="ExternalInput")
    wkT = nc.dram_tensor("wkT", [C, GF], BF16, kind="ExternalInput")
    wvT = nc.dram_tensor("wvT", [C, GF], BF16, kind="ExternalInput")
    wpT = nc.dram_tensor("wpT", [GF, C], BF16, kind="ExternalInput")
    qb = nc.dram_tensor("qb", [GF], F32, kind="ExternalInput")
    pb = nc.dram_tensor("pb", [C], F32, kind="ExternalInput")
    out = nc.dram_tensor("out", [N, C], F32, kind="ExternalOutput")
    aps = (xT, wqT, wkT, wvT, wpT, qb, pb, out)

    with tile.TileContext(nc) as tc:
        with (
            tc.tile_pool(name="const", bufs=1) as const,
            tc.tile_pool(name="qkv", bufs=1) as qkvp,
            tc.tile_pool(name="p", bufs=2) as pp,
            tc.tile_pool(name="norm", bufs=3) as normp,
            tc.tile_pool(name="y", bufs=3) as yp,
            tc.tile_pool(name="psA", bufs=2, space="PSUM") as psA,
            tc.tile_pool(name="psS", bufs=2, space="PSUM") as psS,
            tc.tile_pool(name="psO", bufs=2, space="PSUM") as psO,
        ):
            pools = (const, qkvp, pp, normp, yp, psA, psS, psO)
            for _ in range(krep):
                _body(nc, tc, pools, aps)
    nc.compile()
    return nc


def make_in_maps(x, qkv_weight, q_bias, v_bias, proj_weight, proj_bias):
    bf = ml_dtypes.bfloat16
    f32 = np.float32
    in_maps = []
    for c in range(N_CORES):
        b, g = c // 2, c % 2
        sl = slice(g * GF, (g + 1) * GF)
        wq = np.ascontiguousarray((qkv_weight[sl, :] * SCALE).T).astype(bf)
        wk = np.ascontiguousarray(qkv_weight[C + g * GF : C + (g + 1) * GF, :].T).astype(bf)
        wv = np.ascontiguousarray(qkv_weight[2 * C + g * GF : 2 * C + (g + 1) * GF, :].T).astype(bf)
        wp = np.ascontiguousarray(proj_weight[:, sl].T).astype(bf)
        qb_ = (q_bias[sl] * SCALE).astype(f32)
        vb_ = v_bias[sl].astype(np.float64)
        pb_ = (proj_weight[:, sl].astype(np.float64) @ vb_).astype(f32)
        if g == 0:
            pb_ = (pb_ + proj_bias).astype(f32)
        in_maps.append(
            dict(
                xT=np.ascontiguousarray(x[b].T).astype(bf),
                wqT=wq, wkT=wk, wvT=wv, wpT=wp,
                qb=np.ascontiguousarray(qb_), pb=np.ascontiguousarray(pb_),
            )
        )
    return in_maps


def gather(results):
    out = np.empty((B, N, C), np.float32)
    for b in range(B):
        out[b] = results[2 * b]["out"] + results[2 * b + 1]["out"]
    return out


_NC_CACHE = {}


def kernel(x, qkv_weight, q_bias, v_bias, proj_weight, proj_bias):
    if "nc" not in _NC_CACHE:
        _NC_CACHE["nc"] = build()
    nc = _NC_CACHE["nc"]
    in_maps = make_in_maps(x, qkv_weight, q_bias, v_bias, proj_weight, proj_bias)
    res = run_bass_kernel_spmd(nc, in_maps, core_ids=list(range(N_CORES)))
    return gather(res.results)


if __name__ == "__main__":
    rng = np.random.default_rng(0)
    x = rng.standard_normal((B, N, C), dtype=np.float32)
    qkv_weight = rng.standard_normal((3 * C, C), dtype=np.float32) * C**-0.5
    q_bias = rng.standard_normal(C, dtype=np.float32) * 0.02
    v_bias = rng.standard_normal(C, dtype=np.float32) * 0.02
    proj_weight = rng.standard_normal((C, C), dtype=np.float32) * C**-0.5
    proj_bias = rng.standard_normal(C, dtype=np.float32) * 0.02
    out = kernel(x, qkv_weight, q_bias, v_bias, proj_weight, proj_bias)
    print("out", out.shape, out.dtype, float(np.abs(out).mean()))
